# revision 4
# baseline (speedup 1.0000x reference)
"""DeepFM forward kernel for 8 Trainium2 NeuronCores.

Strategy (data-parallel, per the sharding hint): shard the batch of 2048
rows across 8 cores (256 rows each); replicate the embedding table, FM
linear weight, and MLP params.

v2: replaces the 16 per-(tile,field) indirect DMAs (each ~1.4us of
serialized SWDGE descriptor-gen on GpSimd) with TWO dma_gather
instructions (InstDMAGatherAnt): SWDGE cost is ~1us fixed + 0.34ns per
descriptor, so batching all 2048 descriptors into 2 instructions cuts
~22us of GpSimd serialization to ~2.7us.

dma_gather constraints and how they're met:
  - indices are int16 (max 32767), rows must be 256B-multiple:
    * fields 1-7 (vocab sum 10000): ids-50000 in [0,10000) -> one gather
      from a [10000, 64] f32 table (16 emb + w_lin + pad = 256B rows).
    * field 0 (vocab 50000): ids up to 49999 overflow int16, so rows are
      stored in PAIRS: [25000, 128] f32 (512B = rows 2k,2k+1), gathered
      with idx=id>>1; the right 68B half is selected on-chip with a
      host-precomputed parity mask.
  - index layout: idx g lives at [g%16, g//16] in a [128, n/16] int16
    SBUF tile (first 16 partitions, replicated 8x for the gpsimd cores).

MLP runs in bf16 (weights + activations; f32 PSUM accum) - fp32 PE
matmuls cost 4 passes, bf16 one. FM + linear stay f32.
"""

import numpy as np

import concourse.bass as bass
import concourse.bacc as bacc
import concourse.mybir as mybir
import concourse.tile as tile
from concourse.bass_utils import run_bass_kernel_spmd

N_CORES = 8
B = 2048
BC = B // N_CORES  # 256 rows per core
NT = BC // 128     # 2 tiles of 128 rows
F = 8              # fields
D = 16             # embed dim
FIELD_DIMS = [50000, 5000, 2000, 1000, 1000, 500, 300, 200]
OFFSETS = np.concatenate([[0], np.cumsum(FIELD_DIMS)[:-1]]).astype(np.int64)
INPUT_DIM = int(np.sum(FIELD_DIMS))  # 60000
SPLIT = FIELD_DIMS[0]                # 50000 (field 0 handled separately)
VS = INPUT_DIM - SPLIT               # 10000 small-table rows
VB = SPLIT // 2                      # 25000 big-table pair rows
H1, H2, H3 = 256, 128, 64
NA = NT * (F - 1) * 128              # 1792 small-field gather descriptors
NB = NT * 128                        # 256 field-0 gather descriptors

# blobh (bf16) column layout
HW1 = 0              # w1 [128, 256]
HW2 = HW1 + H1       # w2 chunks [128, 128] x2
HW3 = HW2 + 2 * H2   # w3 [128, 64]
HWL = HW3 + H3       # w_last [64] in partitions 0..63
HID = HWL + 1        # 128x128 identity (PE transpose-by-matmul)
BLOBHW = HID + 128
# blobf (f32) column layout
FB1 = 0              # b1 [256] as 2 cols of 128
FB2 = FB1 + 2        # b2 [128]
FB3 = FB2 + 1        # b3 [64] in partitions 0..63
FBL = FB3 + 1        # b_lin broadcast
BLOBFW = FBL + 1

_CACHE = {}


def _wrap_idx(seq, copies=8):
    """idx g -> [g%16, g//16], replicated to 128 partitions, int16."""
    n = seq.shape[0]
    assert n % 16 == 0
    w = seq.reshape(n // 16, 16).T  # [16, n/16]
    return np.ascontiguousarray(np.tile(w, (copies, 1))).astype(np.int16)


def build_program():
    """Build the single-core Bass/Tile program (SPMD: same NEFF on all cores)."""
    f32 = mybir.dt.float32
    bf16 = mybir.dt.bfloat16
    i16 = mybir.dt.int16
    Alu = mybir.AluOpType
    Act = mybir.ActivationFunctionType

    nc = bacc.Bacc(None, target_bir_lowering=False)
    tabs = nc.dram_tensor("tabs", [VS, 64], f32, kind="ExternalInput")
    tabb = nc.dram_tensor("tabb", [VB, 128], f32, kind="ExternalInput")
    idxa = nc.dram_tensor("idxa", [128, NA // 16], i16, kind="ExternalInput")
    idxb = nc.dram_tensor("idxb", [128, NB // 16], i16, kind="ExternalInput")
    msel = nc.dram_tensor("msel", [128, NT * 17], f32, kind="ExternalInput")
    blobh = nc.dram_tensor("blobh", [128, BLOBHW], bf16, kind="ExternalInput")
    blobf = nc.dram_tensor("blobf", [128, BLOBFW], f32, kind="ExternalInput")
    y = nc.dram_tensor("y", [128, NT], f32, kind="ExternalOutput")

    with tile.TileContext(nc) as tc:
        with (
            tc.tile_pool(name="sb", bufs=2) as sp,
            tc.tile_pool(name="cst", bufs=1) as cp,
            tc.tile_pool(name="ps", bufs=1, space="PSUM") as pp,
        ):
            # index loads first (gathers depend only on these)
            idxa_t = cp.tile([128, NA // 16], i16)
            nc.sync.dma_start(out=idxa_t[:], in_=idxa[:])
            idxb_t = cp.tile([128, NB // 16], i16)
            nc.sync.dma_start(out=idxb_t[:], in_=idxb[:])
            # params on the Activation engine's HWDGE queue (overlaps sync's)
            msel_t = cp.tile([128, NT * 17], f32)
            nc.scalar.dma_start(out=msel_t[:], in_=msel[:])
            blobf_t = cp.tile([128, BLOBFW], f32)
            nc.scalar.dma_start(out=blobf_t[:], in_=blobf[:])
            blobh_t = cp.tile([128, BLOBHW], bf16)
            nc.scalar.dma_start(out=blobh_t[:], in_=blobh[:])

            # the gathers: all 2048 table-row fetches for this core in 3
            # instructions (HW SWDGE carveout caps one call at 1024 descs)
            NH = NA // NT  # 896 descriptors per tile
            gA = cp.tile([128, NT * (F - 1) * 64], f32)
            gA_v = gA[:].rearrange("p (j e) -> p j e", j=NT * (F - 1))
            nc.gpsimd.dma_gather(
                gA_v[:, 0:F - 1, :], tabs[:], idxa_t[:, 0:NH // 16], NH, NH, 64)
            gB = cp.tile([128, NT * 128], f32)
            gB_v = gB[:].rearrange("p (j e) -> p j e", j=NT)
            nc.gpsimd.dma_gather(gB_v, tabb[:], idxb_t[:], NB, NB, 128)
            nc.gpsimd.dma_gather(
                gA_v[:, F - 1:2 * (F - 1), :], tabs[:],
                idxa_t[:, NH // 16:2 * NH // 16], NH, NH, 64)

            # field-0 pair-select, both tiles at once:
            # sel = parity * (row1 - row0); emb0/lin0 = row0 + sel
            g0v = gB_v[:, :, 0:17]
            g1v = gB_v[:, :, 64:81]
            seld = sp.tile([128, NT * 17], f32)
            seld_v = seld[:].rearrange("p (t k) -> p t k", t=NT)
            nc.vector.tensor_sub(out=seld_v, in0=g1v, in1=g0v)
            selm = sp.tile([128, NT * 17], f32)
            selm_v = selm[:].rearrange("p (t k) -> p t k", t=NT)
            nc.vector.tensor_mul(
                out=selm_v, in0=seld_v,
                in1=msel_t[:].rearrange("p (t k) -> p t k", t=NT),
            )

            y_sb = cp.tile([128, NT], f32)
            for i in range(NT):
                ga_i = gA_v[:, i * (F - 1):(i + 1) * (F - 1), :]
                # hc[p, f*16+d]: field 0 from pair-select, fields 1-7 from gA
                hc = sp.tile([128, F * D], f32)
                hc3 = hc[:].rearrange("p (f d) -> p f d", f=F)
                nc.vector.tensor_add(
                    out=hc3[:, 0:1, :],
                    in0=selm_v[:, i:i + 1, 0:16], in1=g0v[:, i:i + 1, 0:16],
                )
                nc.vector.tensor_copy(out=hc3[:, 1:8, :], in_=ga_i[:, :, 0:D])
                # FM linear: w_lin[field0] + sum_f w_lin[fields 1-7]
                lin0 = sp.tile([128, 1], f32)
                nc.vector.tensor_add(
                    out=lin0[:].rearrange("p (t k) -> p t k", t=1),
                    in0=selm_v[:, i:i + 1, 16:17], in1=g0v[:, i:i + 1, 16:17],
                )
                lin7 = sp.tile([128, 1], f32)
                nc.vector.reduce_sum(
                    out=lin7[:], in_=ga_i[:, :, D:D + 1], axis=mybir.AxisListType.XY
                )

                # FM second order: fm2 = (sum_f h)^2 summed - sum h^2
                s4 = sp.tile([128, 4 * D], f32)
                nc.vector.tensor_add(
                    out=s4[:].rearrange("p (f d) -> p f d", f=4),
                    in0=hc3[:, 0:4, :], in1=hc3[:, 4:8, :],
                )
                s43 = s4[:].rearrange("p (f d) -> p f d", f=4)
                s2 = sp.tile([128, 2 * D], f32)
                nc.vector.tensor_add(
                    out=s2[:].rearrange("p (f d) -> p f d", f=2),
                    in0=s43[:, 0:2, :], in1=s43[:, 2:4, :],
                )
                s23 = s2[:].rearrange("p (f d) -> p f d", f=2)
                s1 = sp.tile([128, D], f32)
                nc.vector.tensor_add(
                    out=s1[:].rearrange("p (f d) -> p f d", f=1),
                    in0=s23[:, 0:1, :], in1=s23[:, 1:2, :],
                )
                sq = sp.tile([128, F * D], f32)
                r2 = sp.tile([128, 1], f32)
                nc.scalar.activation(
                    out=sq[:], in_=hc[:], func=Act.Square, accum_out=r2[:],
                )
                ss = sp.tile([128, D], f32)
                r1 = sp.tile([128, 1], f32)
                nc.scalar.activation(
                    out=ss[:], in_=s1[:], func=Act.Square, accum_out=r1[:],
                )
                fm2 = sp.tile([128, 1], f32)
                nc.vector.tensor_sub(out=fm2[:], in0=r1[:], in1=r2[:])

                # MLP in transposed-activation form, bf16 matmuls
                hcb = sp.tile([128, F * D], bf16)
                nc.vector.tensor_copy(out=hcb[:], in_=hc[:])
                hT_p = pp.tile([128, 128], f32)
                nc.tensor.matmul(
                    out=hT_p[:], lhsT=hcb[:], rhs=blobh_t[:, HID:HID + 128],
                    start=True, stop=True,
                )
                hTb = sp.tile([128, 128], bf16)
                nc.vector.tensor_copy(out=hTb[:], in_=hT_p[:])

                a1 = sp.tile([128, H1], bf16)
                for c in range(2):
                    p1 = pp.tile([128, 128], f32)
                    nc.tensor.matmul(
                        out=p1[:],
                        lhsT=blobh_t[:, HW1 + c * 128:HW1 + (c + 1) * 128],
                        rhs=hTb[:], start=True, stop=True,
                    )
                    nc.scalar.activation(
                        out=a1[:, c * 128:(c + 1) * 128], in_=p1[:], func=Act.Relu,
                        bias=blobf_t[:, FB1 + c:FB1 + c + 1], scale=1.0,
                    )
                p2 = pp.tile([128, 128], f32)
                nc.tensor.matmul(
                    out=p2[:], lhsT=blobh_t[:, HW2:HW2 + 128],
                    rhs=a1[:, 0:128], start=True, stop=False,
                )
                nc.tensor.matmul(
                    out=p2[:], lhsT=blobh_t[:, HW2 + 128:HW2 + 256],
                    rhs=a1[:, 128:256], start=False, stop=True,
                )
                a2 = sp.tile([128, H2], bf16)
                nc.scalar.activation(
                    out=a2[:], in_=p2[:], func=Act.Relu,
                    bias=blobf_t[:, FB2:FB2 + 1], scale=1.0,
                )
                p3 = pp.tile([64, 128], f32)
                nc.tensor.matmul(
                    out=p3[:], lhsT=blobh_t[:, HW3:HW3 + H3], rhs=a2[:],
                    start=True, stop=True,
                )
                a3 = sp.tile([64, 128], bf16)
                nc.scalar.activation(
                    out=a3[:], in_=p3[:], func=Act.Relu,
                    bias=blobf_t[0:64, FB3:FB3 + 1], scale=1.0,
                )
                py = pp.tile([128, 1], f32)
                nc.tensor.matmul(
                    out=py[:], lhsT=a3[:], rhs=blobh_t[0:64, HWL:HWL + 1],
                    start=True, stop=True,
                )

                # y = 0.5*fm2 + lin7 + lin0 + b_lin + y_dnn
                t1 = sp.tile([128, 1], f32)
                nc.vector.scalar_tensor_tensor(
                    out=t1[:], in0=fm2[:], scalar=0.5, in1=lin7[:],
                    op0=Alu.mult, op1=Alu.add,
                )
                t2 = sp.tile([128, 1], f32)
                nc.vector.tensor_add(out=t2[:], in0=py[:], in1=blobf_t[:, FBL:FBL + 1])
                t3 = sp.tile([128, 1], f32)
                nc.vector.tensor_add(out=t3[:], in0=t1[:], in1=lin0[:])
                nc.vector.tensor_add(out=y_sb[:, i:i + 1], in0=t2[:], in1=t3[:])

            nc.sync.dma_start(out=y[:], in_=y_sb[:])
    nc.finalize()
    return nc


def prepare_inputs(x, emb_table, w_lin, b_lin, w1, b1, w2, b2, w3, b3, w_last):
    x = np.asarray(x)
    xoff = (x.astype(np.int64) + OFFSETS[None, :]).astype(np.int32)  # [2048, 8]
    emb = np.asarray(emb_table, np.float32)
    wl = np.asarray(w_lin, np.float32)

    tabs = np.zeros((VS, 64), np.float32)
    tabs[:, :D] = emb[SPLIT:]
    tabs[:, D] = wl[SPLIT:]
    tabb = np.zeros((VB, 2, 64), np.float32)
    tabb[:, :, :D] = emb[:SPLIT].reshape(VB, 2, D)
    tabb[:, :, D] = wl[:SPLIT].reshape(VB, 2)
    tabb = tabb.reshape(VB, 128)

    idxa_c, idxb_c, msel_c = [], [], []
    for c in range(N_CORES):
        rows = xoff[c * BC:(c + 1) * BC].reshape(NT, 128, F)  # [t, p, f]
        # gather A: g = (t*(F-1) + (f-1))*128 + p
        seqa = (rows[:, :, 1:] - SPLIT).transpose(0, 2, 1).reshape(NA)
        idxa_c.append(_wrap_idx(seqa))
        id0 = rows[:, :, 0]  # [t, p] field-0 ids (offset 0)
        idxb_c.append(_wrap_idx((id0 >> 1).reshape(NB)))
        m = (id0 & 1).astype(np.float32)  # [t, p]
        msel_c.append(np.repeat(m.T[:, :, None], 17, axis=2).reshape(128, NT * 17))

    blobh = np.zeros((128, BLOBHW), np.float32)
    blobh[:, HW1:HW1 + H1] = np.asarray(w1, np.float32)
    w2 = np.asarray(w2, np.float32)
    blobh[:, HW2:HW2 + H2] = w2[0:128, :]
    blobh[:, HW2 + H2:HW2 + 2 * H2] = w2[128:256, :]
    blobh[:, HW3:HW3 + H3] = np.asarray(w3, np.float32)
    blobh[0:H3, HWL] = np.asarray(w_last, np.float32)[:, 0]
    blobh[:, HID:HID + 128] = np.eye(128, dtype=np.float32)
    import ml_dtypes
    blobh = blobh.astype(ml_dtypes.bfloat16)

    blobf = np.zeros((128, BLOBFW), np.float32)
    b1 = np.asarray(b1, np.float32)
    blobf[:, FB1] = b1[0:128]
    blobf[:, FB1 + 1] = b1[128:256]
    blobf[:, FB2] = np.asarray(b2, np.float32)
    blobf[0:H3, FB3] = np.asarray(b3, np.float32)
    blobf[:, FBL] = np.float32(np.asarray(b_lin))
    return tabs, tabb, blobh, blobf, idxa_c, idxb_c, msel_c


def kernel(**inputs):
    tabs, tabb, blobh, blobf, idxa_c, idxb_c, msel_c = prepare_inputs(**inputs)
    if "nc" not in _CACHE:
        _CACHE["nc"] = build_program()
    nc = _CACHE["nc"]
    in_maps = [
        {"tabs": tabs, "tabb": tabb, "blobh": blobh, "blobf": blobf,
         "idxa": idxa_c[c], "idxb": idxb_c[c], "msel": msel_c[c]}
        for c in range(N_CORES)
    ]
    res = run_bass_kernel_spmd(nc, in_maps, list(range(N_CORES))).results
    # y[c*256 + i*128 + p] = res[c]["y"][p, i]
    out = np.concatenate([res[c]["y"].T.reshape(BC) for c in range(N_CORES)])
    return out.astype(np.float32)


if __name__ == "__main__":
    rng = np.random.default_rng(0)
    demo = {
        "x": np.stack([rng.integers(0, FIELD_DIMS[f], 2048) for f in range(F)], 1).astype(np.int64),
        "emb_table": rng.standard_normal((INPUT_DIM, D), np.float32) * 0.01,
        "w_lin": rng.random(INPUT_DIM, np.float32),
        "b_lin": np.float32(0.0),
        "w1": rng.standard_normal((F * D, H1), np.float32) * 0.1,
        "b1": np.zeros(H1, np.float32),
        "w2": rng.standard_normal((H1, H2), np.float32) * 0.1,
        "b2": np.zeros(H2, np.float32),
        "w3": rng.standard_normal((H2, H3), np.float32) * 0.1,
        "b3": np.zeros(H3, np.float32),
        "w_last": rng.standard_normal((H3, 1), np.float32) * 0.1,
    }
    print(kernel(**demo)[:8])
